# revision 2
# baseline (speedup 1.0000x reference)
"""Trainium2 Bass kernel for nn_AdaptiveAttention (8-core SPMD).

Sharding: each core owns 2 heads (one 128-dim block of the QKV/head space)
for BOTH batches; outputs are resharded by interleaving: core c produces the
q-rows with (row index) % 8 == c.

v2 highlights over the baseline:
  - Q columns are permuted AT THE PROJECTION (matmul rhs AP reads the 512-col
    slab in (slot, pos) order), so the softmax-normalize multiply and the
    AllToAll pushes are fully contiguous -- no strided DVE writes, 4 big
    pushes per q-block instead of 16 small ones.
  - exp() output and V are fp8e4 (scores shifted by -3.5 so et stays in
    fp8 range; the shift cancels exactly in softmax); the AV matmul runs in
    DoubleRow mode (2 key-chunks per pass, 2x PE throughput). V is padded to
    80 dims per (kk,hh) to satisfy the DoubleRow 16B-stride rule; its col 64
    is the ones column producing the exp row-sum in AV output row 64.
  - softmax reciprocal runs directly on the PSUM row-sum row ([1,512] DVE
    op), bounced through DRAM once per q-block for the partition-broadcast
    (SBUF APs cannot have stride-0 partitions).
  - A2A split 3+1 (qb0-2 exchanged after qb2, qb3 alone) so only a small
    collective is exposed in the tail.
  - DMA queues: sync/scalar/gpsimd only (hw limit); scalar queue is kept
    free of DMAs during attention since exp() on the Scalar engine is the
    critical path (~120us floor: 16.7M exps at 128 lanes / 1.2 GHz).
"""

import numpy as np
import ml_dtypes

B, T, D = 2, 2048, 1024
H, DK = 16, 64
CPC = 128               # head-dim columns per core (2 heads)
P = 128
NCORES = 8
SHIFT = -3.5            # exp shift: scores bounded ~[-8.3, 7.9]

_BF16 = ml_dtypes.bfloat16

_CACHE = {}
LAST_RESULTS = None


def _sinusoidal_pe(max_len, d_model):
    pos = np.arange(max_len)[:, None].astype(np.float32)
    div = np.exp(np.arange(0, d_model, 2).astype(np.float32) * (-np.log(10000.0) / d_model))
    pe = np.zeros((max_len, d_model), dtype=np.float32)
    pe[:, 0::2] = np.sin(pos * div)
    pe[:, 1::2] = np.cos(pos * div)
    return pe


def _build():
    """Build + compile the SPMD Bass graph (one NEFF, runs on all 8 cores)."""
    import concourse.bass as bass
    import concourse.mybir as mybir
    import concourse.tile as tile
    from concourse import bacc

    f32 = mybir.dt.float32
    bf = mybir.dt.bfloat16
    f8 = mybir.dt.float8e4
    Exp = mybir.ActivationFunctionType.Exp
    Sqrt = mybir.ActivationFunctionType.Sqrt
    sub = mybir.AluOpType.subtract
    mult = mybir.AluOpType.mult
    DR = mybir.MatmulPerfMode.DoubleRow

    nc = bacc.Bacc("TRN2", target_bir_lowering=False, debug=False, num_devices=NCORES)

    xt_d = nc.dram_tensor("xt", [B, D, T], bf, kind="ExternalInput")
    xres_d = nc.dram_tensor("xres", [4, P, D], f32, kind="ExternalInput")
    wq_d = nc.dram_tensor("wq", [D, CPC], bf, kind="ExternalInput")
    wk_d = nc.dram_tensor("wk", [B, D, CPC], bf, kind="ExternalInput")
    wv_d = nc.dram_tensor("wv", [D, CPC], bf, kind="ExternalInput")
    bq_d = nc.dram_tensor("bq", [B, CPC], f32, kind="ExternalInput")
    bk_d = nc.dram_tensor("bk", [B, CPC], f32, kind="ExternalInput")
    bv_d = nc.dram_tensor("bv", [B, CPC], f32, kind="ExternalInput")
    wo_d = nc.dram_tensor("wo", [D, D], bf, kind="ExternalInput")
    lng_d = nc.dram_tensor("lng", [D], f32, kind="ExternalInput")
    lnb_d = nc.dram_tensor("lnb", [D], f32, kind="ExternalInput")
    out_d = nc.dram_tensor("out", [4, P, D], f32, kind="ExternalOutput")
    # group 0 carries q-blocks 0-2, group 1 only q-block 3
    a2ai_d = [nc.dram_tensor(f"a2ai{i}", [NCORES, CPC, B, g, 64], bf, kind="Internal")
              for i, g in ((0, 3), (1, 1))]
    a2ao_d = [nc.dram_tensor(f"a2ao{i}", [NCORES, CPC, B, g, 64], bf, kind="Internal")
              for i, g in ((0, 3), (1, 1))]
    rsd_d = [nc.dram_tensor(f"rsd{i}", [4, 512], f32, kind="Internal") for i in range(4)]

    def bcast_ap(src, nparts):
        """Partition-broadcast DMA source AP from a 1-partition (DRAM) AP."""
        return bass.AP(
            tensor=src.tensor,
            offset=src.offset,
            ap=[[0, nparts]] + [list(d) for d in src.ap[1:]],
        )

    with tile.TileContext(nc) as tc:
        with tc.tile_pool(name="const", bufs=1) as const:
            qeng = [nc.sync, nc.scalar, nc.gpsimd]
            xt_sb = [[const.tile([P, T], bf, name=f"xt{b}_{k}")
                      for k in range(8)] for b in range(B)]
            xt_ap = xt_d.ap()
            wq_sb = const.tile([P, 8, CPC], bf)
            wk_sb = const.tile([P, B, 8, CPC], bf)
            wv_sb = const.tile([P, 8, CPC], bf)
            for k in range(8):
                qeng[k % 3].dma_start(out=wq_sb[:, k, :], in_=wq_d.ap()[k * P:(k + 1) * P, :])
            for b in range(B):
                for k in range(8):
                    qeng[k % 3].dma_start(
                        out=wk_sb[:, b, k, :], in_=wk_d.ap()[b, k * P:(k + 1) * P, :]
                    )
            for k in range(8):
                qeng[k % 3].dma_start(out=xt_sb[0][k][:], in_=xt_ap[0, k * P:(k + 1) * P, :])
            for k in range(8):
                qeng[k % 3].dma_start(out=xt_sb[1][k][:], in_=xt_ap[1, k * P:(k + 1) * P, :])
            for k in range(8):
                qeng[k % 3].dma_start(out=wv_sb[:, k, :], in_=wv_d.ap()[k * P:(k + 1) * P, :])

            wo_sb = const.tile([P, 8, D], bf)
            wo_ap = wo_d.ap()
            for k in range(8):
                qeng[k % 3].dma_start(out=wo_sb[:, k, :], in_=wo_ap[k * P:(k + 1) * P, :])

            bq_sb = const.tile([P, B], f32)
            bk_sb = const.tile([P, B], f32)
            nc.sync.dma_start(out=bq_sb[:], in_=bq_d.ap().rearrange("b p -> p b"))
            nc.sync.dma_start(out=bk_sb[:], in_=bk_d.ap().rearrange("b p -> p b"))
            bv_bc = const.tile([P, B, CPC], f32)
            for b in range(B):
                nc.sync.dma_start(out=bv_bc[:, b, :], in_=bcast_ap(bv_d.ap()[b:b + 1, :], P))
            lng_b = const.tile([P, D], f32)
            lnb_b = const.tile([P, D], f32)
            nc.sync.dma_start(out=lng_b[:], in_=bcast_ap(lng_d.ap()[None, :], P))
            nc.sync.dma_start(out=lnb_b[:], in_=bcast_ap(lnb_d.ap()[None, :], P))
            xres_sb = const.tile([P, 4, D], f32)
            for qb in range(4):
                nc.scalar.dma_start(out=xres_sb[:, qb, :], in_=xres_d.ap()[qb])

            eps_sb = const.tile([P, 1], f32)
            nc.vector.memset(eps_sb[:], 1e-5)
            shift_sb = const.tile([P, 1], f32)
            nc.vector.memset(shift_sb[:], SHIFT)

            QT_sb = const.tile([P, B, T], bf)       # q-cols permuted (slot-major)
            KT_sb = const.tile([P, B, T], bf)
            # V in fp8, DoubleRow layout: (scp key-pair, kk parity, hh head, 80)
            # col 64 of each 80-slab is the ones column (AV row 64 = exp rowsum);
            # cols 65-79 are zero padding for the 16B-stride DoubleRow rule.
            V8_sb = const.tile([P, B, 8, 2, 2, 80], f8)
            nc.vector.memset(V8_sb[:], 0.0)
            nc.vector.memset(V8_sb[:, :, :, :, :, 64:65], 1.0)
            attT_sb = [const.tile([64, B, 2, 512], bf, name=f"attT{i}")
                       for i in range(4)]

            # ---- phase 1: projections ----
            with tc.tile_pool(name="qk_ps", bufs=4, space="PSUM") as qk_ps, \
                 tc.tile_pool(name="v_ps", bufs=4, space="PSUM") as v_ps:
                for wname, b_sb, dst in (("q", bq_sb, QT_sb), ("k", bk_sb, KT_sb)):
                    for b in range(B):
                        for n in range(4):
                            ps = qk_ps.tile([P, 512], f32, tag="qkps")
                            for k in range(8):
                                lhsT = (wq_sb[:, k, :] if wname == "q"
                                        else wk_sb[:, b, k, :])
                                rhs = xt_sb[b][k][:, n * 512:(n + 1) * 512]
                                if wname == "q":
                                    # fold the mod-8 interleave permute into the
                                    # projection: stream cols in (slot, pos) order
                                    rhs = rhs.rearrange("p (i j) -> p j i", j=8)
                                nc.tensor.matmul(
                                    ps[:], lhsT=lhsT, rhs=rhs,
                                    start=(k == 0), stop=(k == 7),
                                )
                            nc.vector.tensor_scalar_add(
                                out=dst[:, b, n * 512:(n + 1) * 512],
                                in0=ps[:], scalar1=b_sb[:, b:b + 1],
                            )
                for b in range(B):
                    for mt in range(16):
                        ps = v_ps.tile([P, CPC], f32, tag="vps")
                        for k in range(8):
                            nc.tensor.matmul(
                                ps[:],
                                lhsT=xt_sb[b][k][:, mt * P:(mt + 1) * P],
                                rhs=wv_sb[:, k, :],
                                start=(k == 0), stop=(k == 7),
                            )
                        nc.vector.tensor_add(
                            out=V8_sb[:, b, mt // 2, mt % 2, :, 0:64],
                            in0=ps[:].rearrange("p (h d) -> p h d", h=2),
                            in1=bv_bc[:, b, :].rearrange("p (h d) -> p h d", h=2),
                        )

            # ---- phase 2: attention ----
            with tc.tile_pool(name="sp_ps", bufs=2, space="PSUM") as sp_ps, \
                 tc.tile_pool(name="av_ps", bufs=4, space="PSUM") as av_ps, \
                 tc.tile_pool(name="et_pool", bufs=3) as et_pool, \
                 tc.tile_pool(name="rc_pool", bufs=8) as rc_pool, \
                 tc.tile_pool(name="rb_pool", bufs=2) as rb_pool:
                for qb in range(4):
                    qsl = slice(qb * 512, (qb + 1) * 512)
                    g, off = (0, qb) if qb < 3 else (1, 0)
                    avs = [av_ps.tile([80, 512], f32, tag="av", name=f"av{qb}_{u}")
                           for u in range(4)]
                    for b in range(B):
                        et = None
                        for sc in range(16):
                            sp = sp_ps.tile([P, 2, 512], f32, tag="sp")
                            nc.tensor.matmul(
                                sp[:, 0, :],
                                lhsT=KT_sb[0:64, b, sc * P:(sc + 1) * P],
                                rhs=QT_sb[0:64, b, qsl],
                                start=True, stop=True,
                            )
                            nc.tensor.matmul(
                                sp[:, 1, :],
                                lhsT=KT_sb[64:128, b, sc * P:(sc + 1) * P],
                                rhs=QT_sb[64:128, b, qsl],
                                start=True, stop=True,
                            )
                            if sc % 2 == 0:
                                et = et_pool.tile([P, 2, 2, 512], f8, tag="et")
                            nc.scalar.activation(
                                out=et[:, sc % 2, :, :], in_=sp[:],
                                func=Exp, bias=shift_sb[:],
                            )
                            if sc % 2 == 1:
                                for hh in range(2):
                                    nc.tensor.matmul(
                                        avs[2 * b + hh][:],
                                        lhsT=V8_sb[:, b, sc // 2, :, hh, :],
                                        rhs=et[:, :, hh, :],
                                        start=(sc == 1), stop=(sc == 15),
                                        perf_mode=DR,
                                    )
                    # epilogue: reciprocal straight off the PSUM rowsum row,
                    # DRAM-bounce broadcast, normalize, contiguous pushes.
                    rcs = []
                    for u in range(4):
                        rc = rc_pool.tile([1, 512], f32, tag="rc", name=f"rc{qb}_{u}")
                        nc.vector.reciprocal(out=rc[:], in_=avs[u][64:65, :])
                        nc.gpsimd.dma_start(out=rsd_d[qb].ap()[u:u + 1, :], in_=rc[:])
                        rcs.append(rc)
                    rcb = rb_pool.tile([64, 4, 512], f32, tag="rcb")
                    nc.sync.dma_start(out=rcb[:], in_=bcast_ap(rsd_d[qb].ap()[None], 64))
                    for b in range(B):
                        for hh in range(2):
                            u = 2 * b + hh
                            nc.vector.tensor_mul(
                                out=attT_sb[qb][:, b, hh, :],
                                in0=avs[u][0:64, :],
                                in1=rcb[:, u, :],
                            )
                            nc.gpsimd.dma_start(
                                out=a2ai_d[g].ap()[:, hh * 64:(hh + 1) * 64, b, off, :]
                                    .rearrange("j p i -> p j i"),
                                in_=attT_sb[qb][:, b, hh, :]
                                    .rearrange("p (j i) -> p j i", j=8),
                            )
                    if qb == 2:
                        nc.gpsimd.collective_compute(
                            "AllToAll",
                            mybir.AluOpType.bypass,
                            replica_groups=[list(range(NCORES))],
                            ins=[a2ai_d[0].ap()],
                            outs=[a2ao_d[0].ap()],
                        )
                    if qb == 3:
                        nc.gpsimd.collective_compute(
                            "AllToAll",
                            mybir.AluOpType.bypass,
                            replica_groups=[list(range(NCORES))],
                            ins=[a2ai_d[1].ap()],
                            outs=[a2ao_d[1].ap()],
                        )

            # ---- phase 3: interleaved-row Wo + residual + LN per q-block ----
            with tc.tile_pool(name="wo_ps", bufs=2, space="PSUM") as wo_ps, \
                 tc.tile_pool(name="attf_pool", bufs=2) as attf_pool, \
                 tc.tile_pool(name="y_pool", bufs=2) as y_pool, \
                 tc.tile_pool(name="stat", bufs=4) as stat:
                for qb in range(4):
                    g, off = (0, qb) if qb < 3 else (1, 0)
                    attf = attf_pool.tile([P, 8, B, 64], bf, tag="attf")
                    for k in range(8):
                        eng = nc.sync if k % 2 == 0 else nc.gpsimd
                        eng.dma_start(
                            out=attf[:, k, :, :],
                            in_=a2ao_d[g].ap()[k][:, :, off, :],
                        )
                    y = y_pool.tile([P, D], f32, tag="y")
                    for n in range(2):
                        nsl = slice(n * 512, (n + 1) * 512)
                        ps = wo_ps.tile([P, 512], f32, tag="wops")
                        for k in range(8):
                            nc.tensor.matmul(
                                ps[:],
                                lhsT=attf[:, k, :, :],
                                rhs=wo_sb[:, k, nsl],
                                start=(k == 0), stop=(k == 7),
                            )
                        nc.vector.tensor_add(out=y[:, nsl], in0=ps[:], in1=xres_sb[:, qb, nsl])
                    st = stat.tile([P, 2, 6], f32, tag="st")
                    nc.vector.bn_stats(out=st[:, 0, :], in_=y[:, 0:512])
                    nc.vector.bn_stats(out=st[:, 1, :], in_=y[:, 512:1024])
                    mv = stat.tile([P, 2], f32, tag="mv")
                    nc.vector.bn_aggr(out=mv[:], in_=st[:])
                    std = stat.tile([P, 1], f32, tag="std")
                    nc.scalar.activation(out=std[:], in_=mv[:, 1:2], func=Sqrt, bias=eps_sb[:])
                    rstd = stat.tile([P, 1], f32, tag="rstd")
                    nc.vector.reciprocal(out=rstd[:], in_=std[:])
                    nc.vector.tensor_scalar(
                        out=y[:], in0=y[:], scalar1=mv[:, 0:1], scalar2=rstd[:],
                        op0=sub, op1=mult,
                    )
                    nc.vector.tensor_mul(out=y[:], in0=y[:], in1=lng_b[:])
                    nc.vector.tensor_add(out=y[:], in0=y[:], in1=lnb_b[:])
                    nc.sync.dma_start(out=out_d.ap()[qb], in_=y[:])

    nc.compile()
    return nc


def _prep_inputs(x, quantile, quantile_importance,
                 Wq, bq, Wk, bk, Wv, bv, Wo, bo,
                 qpq_w1, qpq_b1, qpq_w2, qpq_b2,
                 qpk_w1, qpk_b1, qpk_w2, qpk_b2,
                 qpv_w1, qpv_b1, qpv_w2, qpv_b2,
                 ln_g, ln_b):
    pe = _sinusoidal_pe(T, D)
    xp = x.astype(np.float32) + pe[None]

    q = quantile.astype(np.float32)

    def mlp(w1, b1, w2, b2):
        return np.maximum(q @ w1 + b1, 0.0) @ w2 + b2

    q_embed = mlp(qpq_w1, qpq_b1, qpq_w2, qpq_b2)
    k_embed = mlp(qpk_w1, qpk_b1, qpk_w2, qpk_b2)
    v_embed = mlp(qpv_w1, qpv_b1, qpv_w2, qpv_b2)

    buf = quantile_importance.astype(np.float32)
    idx = np.clip((q[:, 0] * 100).astype(np.int32), 0, 99)
    mx = buf.max()
    imp = buf[idx]
    imp = np.where(mx > 0, imp / mx, imp)
    scales = (1.0 + imp).astype(np.float32)          # [B], folded into Wk/bk
    rdk = np.float32(1.0 / np.sqrt(DK))              # folded into Wq/bq

    xt_all = np.ascontiguousarray(np.transpose(xp, (0, 2, 1))).astype(_BF16)  # [B, D, T]
    xpb = xp + bo[None, None, :]
    in_maps = []
    for c in range(NCORES):
        cols = slice(c * CPC, (c + 1) * CPC)
        # interleaved residual rows: xres[qb, b*64+jj] = (x+pe+bo)[b, qb*512 + jj*8 + c]
        xres = np.empty((4, P, D), np.float32)
        for qbi in range(4):
            for b in range(B):
                xres[qbi, b * 64:(b + 1) * 64] = xpb[b, qbi * 512 + c: (qbi + 1) * 512: 8]
        in_maps.append({
            "xt": xt_all,
            "xres": xres,
            "wq": np.ascontiguousarray(Wq[:, cols] * rdk).astype(_BF16),
            "wk": np.ascontiguousarray(Wk[None, :, cols] * scales[:, None, None]).astype(_BF16),
            "wv": np.ascontiguousarray(Wv[:, cols]).astype(_BF16),
            "bq": np.ascontiguousarray((bq[None, cols] + q_embed[:, cols]) * rdk).astype(np.float32),
            "bk": np.ascontiguousarray((bk[None, cols] + k_embed[:, cols]) * scales[:, None]).astype(np.float32),
            "bv": np.ascontiguousarray(bv[None, cols] + v_embed[:, cols]).astype(np.float32),
            "wo": Wo.astype(_BF16),
            "lng": ln_g.astype(np.float32),
            "lnb": ln_b.astype(np.float32),
        })
    return in_maps


def kernel(**inputs):
    global LAST_RESULTS
    from concourse import bass_utils

    inputs = {k: np.asarray(v) for k, v in inputs.items()}
    if "nc" not in _CACHE:
        _CACHE["nc"] = _build()
    nc = _CACHE["nc"]

    in_maps = _prep_inputs(**inputs)
    res = bass_utils.run_bass_kernel_spmd(nc, in_maps, core_ids=list(range(NCORES)))
    LAST_RESULTS = res

    out = np.zeros((B, T, D), np.float32)
    for c in range(NCORES):
        o = res.results[c]["out"]  # [4, 128, D]; row = b*64+jj
        for qbi in range(4):
            for b in range(B):
                out[b, qbi * 512 + c:(qbi + 1) * 512:8, :] = o[qbi, b * 64:(b + 1) * 64]
    return out


# revision 10
# speedup vs baseline: 1.0883x; 1.0883x over previous
"""Trainium2 Bass kernel for nn_AdaptiveAttention (8-core SPMD).

Sharding: each core owns 2 heads (one 128-dim block of the QKV/head space)
for BOTH batches; outputs are resharded in contiguous 64-row blocks: within
q-block qb, core c produces q-rows [qb*512 + 64*c, qb*512 + 64*(c+1)).

v2 highlights over the baseline:
  - block (not mod-8 interleaved) q-row ownership, so the softmax-normalize
    multiply and the AllToAll pushes are fully contiguous -- no strided DVE
    writes or permutes anywhere, 4 big pushes per q-block instead of 16
    small ones.
  - exp() output and V are fp8e4 (scores shifted by -3.5 so et stays in
    fp8 range; the shift cancels exactly in softmax); the AV matmul runs in
    DoubleRow mode (2 key-chunks per pass, 2x PE throughput). V is padded to
    80 dims per (kk,hh) to satisfy the DoubleRow 16B-stride rule; its col 64
    is the ones column producing the exp row-sum in AV output row 64.
  - softmax reciprocal runs directly on the PSUM row-sum row ([1,512] DVE
    op), bounced through DRAM once per q-block for the partition-broadcast
    (SBUF APs cannot have stride-0 partitions).
  - A2A split 3+1 (qb0-2 exchanged after qb2, qb3 alone) so only a small
    collective is exposed in the tail.
  - DMA queues: sync/scalar/gpsimd only (hw limit); scalar queue is kept
    free of DMAs during attention since exp() on the Scalar engine is the
    critical path (~120us floor: 16.7M exps at 128 lanes / 1.2 GHz).
"""

import numpy as np
import ml_dtypes

B, T, D = 2, 2048, 1024
H, DK = 16, 64
CPC = 128               # head-dim columns per core (2 heads)
P = 128
NCORES = 8
SHIFT = -3.5            # exp shift: scores bounded ~[-8.3, 7.9]

_BF16 = ml_dtypes.bfloat16

_CACHE = {}
LAST_RESULTS = None


def _sinusoidal_pe(max_len, d_model):
    pos = np.arange(max_len)[:, None].astype(np.float32)
    div = np.exp(np.arange(0, d_model, 2).astype(np.float32) * (-np.log(10000.0) / d_model))
    pe = np.zeros((max_len, d_model), dtype=np.float32)
    pe[:, 0::2] = np.sin(pos * div)
    pe[:, 1::2] = np.cos(pos * div)
    return pe


def _build():
    """Build + compile the SPMD Bass graph (one NEFF, runs on all 8 cores)."""
    import concourse.bass as bass
    import concourse.mybir as mybir
    import concourse.tile as tile
    from concourse import bacc

    f32 = mybir.dt.float32
    bf = mybir.dt.bfloat16
    f8 = mybir.dt.float8e4
    Exp = mybir.ActivationFunctionType.Exp
    Sqrt = mybir.ActivationFunctionType.Sqrt
    sub = mybir.AluOpType.subtract
    mult = mybir.AluOpType.mult
    DR = mybir.MatmulPerfMode.DoubleRow

    nc = bacc.Bacc("TRN2", target_bir_lowering=False, debug=False, num_devices=NCORES)

    xt_d = nc.dram_tensor("xt", [B, D, T], bf, kind="ExternalInput")
    xres_d = nc.dram_tensor("xres", [4, P, D], f32, kind="ExternalInput")
    wq_d = nc.dram_tensor("wq", [D, CPC], bf, kind="ExternalInput")
    wk_d = nc.dram_tensor("wk", [B, D, CPC], bf, kind="ExternalInput")
    wv_d = nc.dram_tensor("wv", [D, CPC], bf, kind="ExternalInput")
    bq_d = nc.dram_tensor("bq", [B, CPC], f32, kind="ExternalInput")
    bk_d = nc.dram_tensor("bk", [B, CPC], f32, kind="ExternalInput")
    bv_d = nc.dram_tensor("bv", [B, CPC], f32, kind="ExternalInput")
    wo_d = nc.dram_tensor("wo", [D, D], bf, kind="ExternalInput")
    lng_d = nc.dram_tensor("lng", [D], f32, kind="ExternalInput")
    lnb_d = nc.dram_tensor("lnb", [D], f32, kind="ExternalInput")
    out_d = nc.dram_tensor("out", [4, P, D], f32, kind="ExternalOutput")
    # group 0 carries q-blocks 0-2, group 1 only q-block 3
    a2ai_d = [nc.dram_tensor(f"a2ai{i}", [NCORES, CPC, B, g, 64], bf, kind="Internal")
              for i, g in ((0, 3), (1, 1))]
    a2ao_d = [nc.dram_tensor(f"a2ao{i}", [NCORES, CPC, B, g, 64], bf, kind="Internal")
              for i, g in ((0, 3), (1, 1))]
    rsd_d = [nc.dram_tensor(f"rsd{i}", [4, 512], f32, kind="Internal") for i in range(4)]

    def bcast_ap(src, nparts):
        """Partition-broadcast DMA source AP from a 1-partition (DRAM) AP."""
        return bass.AP(
            tensor=src.tensor,
            offset=src.offset,
            ap=[[0, nparts]] + [list(d) for d in src.ap[1:]],
        )

    with tile.TileContext(nc) as tc:
        with tc.tile_pool(name="const", bufs=1) as const:
            qeng = [nc.sync, nc.scalar, nc.gpsimd]
            xt_sb = [[const.tile([P, T], bf, name=f"xt{b}_{k}")
                      for k in range(8)] for b in range(B)]
            xt_ap = xt_d.ap()
            wq_sb = const.tile([P, 8, CPC], bf)
            wk_sb = const.tile([P, B, 8, CPC], bf)
            wv_sb = const.tile([P, 8, CPC], bf)
            for k in range(8):
                qeng[k % 3].dma_start(out=wq_sb[:, k, :], in_=wq_d.ap()[k * P:(k + 1) * P, :])
            for b in range(B):
                for k in range(8):
                    qeng[k % 3].dma_start(
                        out=wk_sb[:, b, k, :], in_=wk_d.ap()[b, k * P:(k + 1) * P, :]
                    )
            for k in range(8):
                qeng[k % 3].dma_start(out=xt_sb[0][k][:], in_=xt_ap[0, k * P:(k + 1) * P, :])
            for k in range(8):
                qeng[k % 3].dma_start(out=xt_sb[1][k][:], in_=xt_ap[1, k * P:(k + 1) * P, :])
            for k in range(8):
                qeng[k % 3].dma_start(out=wv_sb[:, k, :], in_=wv_d.ap()[k * P:(k + 1) * P, :])

            wo_sb = const.tile([P, 8, D], bf)
            wo_ap = wo_d.ap()
            for k in range(8):
                qeng[k % 3].dma_start(out=wo_sb[:, k, :], in_=wo_ap[k * P:(k + 1) * P, :])

            bq_sb = const.tile([P, B], f32)
            bk_sb = const.tile([P, B], f32)
            nc.sync.dma_start(out=bq_sb[:], in_=bq_d.ap().rearrange("b p -> p b"))
            nc.sync.dma_start(out=bk_sb[:], in_=bk_d.ap().rearrange("b p -> p b"))
            bv_bc = const.tile([P, B, CPC], f32)
            for b in range(B):
                nc.sync.dma_start(out=bv_bc[:, b, :], in_=bcast_ap(bv_d.ap()[b:b + 1, :], P))
            lng_b = const.tile([P, D], f32)
            lnb_b = const.tile([P, D], f32)
            nc.sync.dma_start(out=lng_b[:], in_=bcast_ap(lng_d.ap()[None, :], P))
            nc.sync.dma_start(out=lnb_b[:], in_=bcast_ap(lnb_d.ap()[None, :], P))
            xres_sb = const.tile([P, 4, D], f32)
            for qb in range(4):
                nc.scalar.dma_start(out=xres_sb[:, qb, :], in_=xres_d.ap()[qb])

            eps_sb = const.tile([P, 1], f32)
            nc.vector.memset(eps_sb[:], 1e-5)
            shift_sb = const.tile([P, 1], f32)
            nc.vector.memset(shift_sb[:], SHIFT)

            QT_sb = const.tile([P, B, T], bf)       # q-cols permuted (slot-major)
            KT_sb = const.tile([P, B, T], bf)
            # V in fp8, DoubleRow layout: (scp key-pair, kk parity, hh head, 80)
            # col 64 of each 80-slab is the ones column (AV row 64 = exp rowsum);
            # cols 65-79 are zero padding for the 16B-stride DoubleRow rule.
            V8_sb = const.tile([P, B, 8, 2, 2, 80], f8)
            nc.vector.memset(V8_sb[:], 0.0)
            nc.vector.memset(V8_sb[:, :, :, :, :, 64:65], 1.0)
            attT_sb = [const.tile([64, B, 2, 512], bf, name=f"attT{i}")
                       for i in range(4)]

            # ---- phase 1: projections ----
            with tc.tile_pool(name="qk_ps", bufs=4, space="PSUM") as qk_ps, \
                 tc.tile_pool(name="v_ps", bufs=4, space="PSUM") as v_ps:
                for wname, b_sb, dst in (("q", bq_sb, QT_sb), ("k", bk_sb, KT_sb)):
                    for b in range(B):
                        for n in range(4):
                            ps = qk_ps.tile([P, 512], f32, tag="qkps")
                            for k in range(8):
                                lhsT = (wq_sb[:, k, :] if wname == "q"
                                        else wk_sb[:, b, k, :])
                                nc.tensor.matmul(
                                    ps[:], lhsT=lhsT,
                                    rhs=xt_sb[b][k][:, n * 512:(n + 1) * 512],
                                    start=(k == 0), stop=(k == 7),
                                )
                            nc.vector.tensor_scalar_add(
                                out=dst[:, b, n * 512:(n + 1) * 512],
                                in0=ps[:], scalar1=b_sb[:, b:b + 1],
                            )
                for b in range(B):
                    for mt in range(16):
                        ps = v_ps.tile([P, CPC], f32, tag="vps")
                        for k in range(8):
                            nc.tensor.matmul(
                                ps[:],
                                lhsT=xt_sb[b][k][:, mt * P:(mt + 1) * P],
                                rhs=wv_sb[:, k, :],
                                start=(k == 0), stop=(k == 7),
                            )
                        nc.vector.tensor_add(
                            out=V8_sb[:, b, mt // 2, mt % 2, :, 0:64],
                            in0=ps[:].rearrange("p (h d) -> p h d", h=2),
                            in1=bv_bc[:, b, :].rearrange("p (h d) -> p h d", h=2),
                        )

            # ---- phase 2: attention ----
            with tc.tile_pool(name="sp_ps", bufs=2, space="PSUM") as sp_ps, \
                 tc.tile_pool(name="av_ps", bufs=4, space="PSUM") as av_ps, \
                 tc.tile_pool(name="et_pool", bufs=3) as et_pool, \
                 tc.tile_pool(name="rc_pool", bufs=2) as rc_pool, \
                 tc.tile_pool(name="rb_pool", bufs=2) as rb_pool:
                for qb in range(4):
                    qsl = slice(qb * 512, (qb + 1) * 512)
                    g, off = (0, qb) if qb < 3 else (1, 0)
                    avs = [av_ps.tile([80, 512], f32, tag="av", name=f"av{qb}_{u}")
                           for u in range(4)]
                    for b in range(B):
                        et = None
                        for sc in range(16):
                            sp = sp_ps.tile([P, 2, 512], f32, tag="sp")
                            nc.tensor.matmul(
                                sp[:, 0, :],
                                lhsT=KT_sb[0:64, b, sc * P:(sc + 1) * P],
                                rhs=QT_sb[0:64, b, qsl],
                                start=True, stop=True,
                            )
                            nc.tensor.matmul(
                                sp[:, 1, :],
                                lhsT=KT_sb[64:128, b, sc * P:(sc + 1) * P],
                                rhs=QT_sb[64:128, b, qsl],
                                start=True, stop=True,
                            )
                            if sc % 2 == 0:
                                et = et_pool.tile([P, 2, 2, 512], f8, tag="et")
                            nc.scalar.activation(
                                out=et[:, sc % 2, :, :], in_=sp[:],
                                func=Exp, bias=shift_sb[:],
                            )
                            if sc % 2 == 1:
                                for hh in range(2):
                                    nc.tensor.matmul(
                                        avs[2 * b + hh][:],
                                        lhsT=V8_sb[:, b, sc // 2, :, hh, :],
                                        rhs=et[:, :, hh, :],
                                        start=(sc == 1), stop=(sc == 15),
                                        perf_mode=DR,
                                    )
                    # epilogue: reciprocal straight off the PSUM rowsum row,
                    # DRAM-bounce broadcast, normalize, contiguous pushes.
                    rs4 = rc_pool.tile([1, 4, 512], f32, tag="rs4", name=f"rs4_{qb}")
                    rc4 = rc_pool.tile([1, 4, 512], f32, tag="rc4", name=f"rc4_{qb}")
                    for u in range(4):
                        nc.vector.tensor_copy(out=rs4[:, u, :], in_=avs[u][64:65, :])
                    # approx_fast needs SBUF input (PSUM reads give garbage)
                    nc.vector.reciprocal_approx_fast(out=rc4[:], in_=rs4[:])
                    nc.gpsimd.dma_start(out=rsd_d[qb].ap()[None], in_=rc4[:])
                    rcb = rb_pool.tile([64, 4, 512], f32, tag="rcb")
                    nc.sync.dma_start(out=rcb[:], in_=bcast_ap(rsd_d[qb].ap()[None], 64))
                    for b in range(B):
                        for hh in range(2):
                            u = 2 * b + hh
                            nc.vector.tensor_mul(
                                out=attT_sb[qb][:, b, hh, :],
                                in0=avs[u][0:64, :],
                                in1=rcb[:, u, :],
                            )
                            nc.gpsimd.dma_start(
                                out=a2ai_d[g].ap()[:, hh * 64:(hh + 1) * 64, b, off, :]
                                    .rearrange("j p i -> p j i"),
                                in_=attT_sb[qb][:, b, hh, :]
                                    .rearrange("p (j i) -> p j i", j=8),
                            )
                    if qb == 2:
                        nc.gpsimd.collective_compute(
                            "AllToAll",
                            mybir.AluOpType.bypass,
                            replica_groups=[list(range(NCORES))],
                            ins=[a2ai_d[0].ap()],
                            outs=[a2ao_d[0].ap()],
                        )
                    if qb == 3:
                        nc.gpsimd.collective_compute(
                            "AllToAll",
                            mybir.AluOpType.bypass,
                            replica_groups=[list(range(NCORES))],
                            ins=[a2ai_d[1].ap()],
                            outs=[a2ao_d[1].ap()],
                        )

            # ---- phase 3: interleaved-row Wo + residual + LN per q-block ----
            with tc.tile_pool(name="wo_ps", bufs=2, space="PSUM") as wo_ps, \
                 tc.tile_pool(name="attf_pool", bufs=2) as attf_pool, \
                 tc.tile_pool(name="y_pool", bufs=2) as y_pool, \
                 tc.tile_pool(name="stat", bufs=4) as stat:
                for qb in range(4):
                    g, off = (0, qb) if qb < 3 else (1, 0)
                    attf = attf_pool.tile([P, 8, B, 64], bf, tag="attf")
                    for k in range(8):
                        eng = nc.sync if k % 2 == 0 else nc.gpsimd
                        eng.dma_start(
                            out=attf[:, k, :, :],
                            in_=a2ao_d[g].ap()[k][:, :, off, :],
                        )
                    y = y_pool.tile([P, D], f32, tag="y")
                    for n in range(2):
                        nsl = slice(n * 512, (n + 1) * 512)
                        ps = wo_ps.tile([P, 512], f32, tag="wops")
                        for k in range(8):
                            nc.tensor.matmul(
                                ps[:],
                                lhsT=attf[:, k, :, :],
                                rhs=wo_sb[:, k, nsl],
                                start=(k == 0), stop=(k == 7),
                            )
                        nc.vector.tensor_add(out=y[:, nsl], in0=ps[:], in1=xres_sb[:, qb, nsl])
                    st = stat.tile([P, 2, 6], f32, tag="st")
                    nc.vector.bn_stats(out=st[:, 0, :], in_=y[:, 0:512])
                    nc.vector.bn_stats(out=st[:, 1, :], in_=y[:, 512:1024])
                    mv = stat.tile([P, 2], f32, tag="mv")
                    nc.vector.bn_aggr(out=mv[:], in_=st[:])
                    std = stat.tile([P, 1], f32, tag="std")
                    nc.scalar.activation(out=std[:], in_=mv[:, 1:2], func=Sqrt, bias=eps_sb[:])
                    rstd = stat.tile([P, 1], f32, tag="rstd")
                    nc.vector.reciprocal(out=rstd[:], in_=std[:])
                    nc.vector.tensor_scalar(
                        out=y[:], in0=y[:], scalar1=mv[:, 0:1], scalar2=rstd[:],
                        op0=sub, op1=mult,
                    )
                    nc.vector.tensor_mul(out=y[:], in0=y[:], in1=lng_b[:])
                    nc.vector.tensor_add(out=y[:], in0=y[:], in1=lnb_b[:])
                    nc.sync.dma_start(out=out_d.ap()[qb], in_=y[:])

    nc.compile()
    return nc


def _prep_inputs(x, quantile, quantile_importance,
                 Wq, bq, Wk, bk, Wv, bv, Wo, bo,
                 qpq_w1, qpq_b1, qpq_w2, qpq_b2,
                 qpk_w1, qpk_b1, qpk_w2, qpk_b2,
                 qpv_w1, qpv_b1, qpv_w2, qpv_b2,
                 ln_g, ln_b):
    pe = _sinusoidal_pe(T, D)
    xp = x.astype(np.float32) + pe[None]

    q = quantile.astype(np.float32)

    def mlp(w1, b1, w2, b2):
        return np.maximum(q @ w1 + b1, 0.0) @ w2 + b2

    q_embed = mlp(qpq_w1, qpq_b1, qpq_w2, qpq_b2)
    k_embed = mlp(qpk_w1, qpk_b1, qpk_w2, qpk_b2)
    v_embed = mlp(qpv_w1, qpv_b1, qpv_w2, qpv_b2)

    buf = quantile_importance.astype(np.float32)
    idx = np.clip((q[:, 0] * 100).astype(np.int32), 0, 99)
    mx = buf.max()
    imp = buf[idx]
    imp = np.where(mx > 0, imp / mx, imp)
    scales = (1.0 + imp).astype(np.float32)          # [B], folded into Wk/bk
    rdk = np.float32(1.0 / np.sqrt(DK))              # folded into Wq/bq

    xt_all = np.ascontiguousarray(np.transpose(xp, (0, 2, 1))).astype(_BF16)  # [B, D, T]
    xpb = xp + bo[None, None, :]
    in_maps = []
    for c in range(NCORES):
        cols = slice(c * CPC, (c + 1) * CPC)
        # block residual rows: xres[qb, b*64+i] = (x+pe+bo)[b, qb*512 + 64*c + i]
        xres = np.empty((4, P, D), np.float32)
        for qbi in range(4):
            for b in range(B):
                base = qbi * 512 + 64 * c
                xres[qbi, b * 64:(b + 1) * 64] = xpb[b, base:base + 64]
        in_maps.append({
            "xt": xt_all,
            "xres": xres,
            "wq": np.ascontiguousarray(Wq[:, cols] * rdk).astype(_BF16),
            "wk": np.ascontiguousarray(Wk[None, :, cols] * scales[:, None, None]).astype(_BF16),
            "wv": np.ascontiguousarray(Wv[:, cols]).astype(_BF16),
            "bq": np.ascontiguousarray((bq[None, cols] + q_embed[:, cols]) * rdk).astype(np.float32),
            "bk": np.ascontiguousarray((bk[None, cols] + k_embed[:, cols]) * scales[:, None]).astype(np.float32),
            "bv": np.ascontiguousarray(bv[None, cols] + v_embed[:, cols]).astype(np.float32),
            "wo": Wo.astype(_BF16),
            "lng": ln_g.astype(np.float32),
            "lnb": ln_b.astype(np.float32),
        })
    return in_maps


def kernel(**inputs):
    global LAST_RESULTS
    from concourse import bass_utils

    inputs = {k: np.asarray(v) for k, v in inputs.items()}
    if "nc" not in _CACHE:
        _CACHE["nc"] = _build()
    nc = _CACHE["nc"]

    in_maps = _prep_inputs(**inputs)
    res = bass_utils.run_bass_kernel_spmd(nc, in_maps, core_ids=list(range(NCORES)))
    LAST_RESULTS = res

    out = np.zeros((B, T, D), np.float32)
    for c in range(NCORES):
        o = res.results[c]["out"]  # [4, 128, D]; row = b*64+i
        for qbi in range(4):
            for b in range(B):
                base = qbi * 512 + 64 * c
                out[b, base:base + 64, :] = o[qbi, b * 64:(b + 1) * 64]
    return out


# revision 11
# speedup vs baseline: 1.2602x; 1.1580x over previous
"""Trainium2 Bass kernel for nn_AdaptiveAttention (8-core SPMD).

Sharding: each core owns 2 heads (one 128-dim block of the QKV/head space)
for BOTH batches; outputs are resharded in contiguous 64-row blocks: within
q-block qb, core c produces q-rows [qb*512 + 64*c, qb*512 + 64*(c+1)).

v3 highlights:
  - block (not interleaved) q-row ownership: normalize multiply and A2A
    pushes are fully contiguous, 4 big pushes per q-block.
  - x and the QKV weights travel in fp8e4 (weights host-prescaled by 64/16
    to dodge fp8 subnormals; the inverse scale rides the PSUM drain), and
    all three projections run DoubleRow (2 key-chunks per matmul pass).
    Halves both the x DMA lead-in (4MB) and the projection PE time.
  - exp() output is fp8e4 (scores shifted by -3.5; shift cancels in
    softmax); V fp8 padded to 80 dims (DoubleRow 16B-stride rule) with a
    ones column so AV row 64 is the exp row-sum; AV is DoubleRow too.
  - softmax reciprocal via reciprocal_approx_fast on an SBUF copy of the
    rowsum rows (PSUM input gives garbage; exact reciprocal on 1 partition
    costs 3.3us); partition-broadcast via one DRAM bounce per (qb, b).
  - epilogues are split per (qb, b) so the b0 half hides under b1's
    attention; one AllToAll per q-block (4 total) so cores re-align four
    times and only a 256KB exchange is exposed in the tail.
  - Scalar engine runs only the exps during attention (its 16.7M exps at
    128 lanes / 1.2GHz = ~118us are the floor); phase-3 LN tail ops are
    split between DVE and the Pool engine, attf loads use the then-idle
    scalar DMA queue.
"""

import numpy as np
import ml_dtypes

B, T, D = 2, 2048, 1024
H, DK = 16, 64
CPC = 128               # head-dim columns per core (2 heads)
P = 128
NCORES = 8
SHIFT = -3.5            # exp shift: scores bounded ~[-8.3, 7.9]
WQ_SCALE = 64.0         # host premultiplier on wq (fp8 subnormal dodge)
WKV_SCALE = 16.0        # host premultiplier on wk/wv

_BF16 = ml_dtypes.bfloat16
_F8 = ml_dtypes.float8_e4m3

_CACHE = {}
LAST_RESULTS = None


def _sinusoidal_pe(max_len, d_model):
    pos = np.arange(max_len)[:, None].astype(np.float32)
    div = np.exp(np.arange(0, d_model, 2).astype(np.float32) * (-np.log(10000.0) / d_model))
    pe = np.zeros((max_len, d_model), dtype=np.float32)
    pe[:, 0::2] = np.sin(pos * div)
    pe[:, 1::2] = np.cos(pos * div)
    return pe


def _build():
    """Build + compile the SPMD Bass graph (one NEFF, runs on all 8 cores)."""
    import concourse.bass as bass
    import concourse.mybir as mybir
    import concourse.tile as tile
    from concourse import bacc

    f32 = mybir.dt.float32
    bf = mybir.dt.bfloat16
    f8 = mybir.dt.float8e4
    Exp = mybir.ActivationFunctionType.Exp
    Sqrt = mybir.ActivationFunctionType.Sqrt
    sub = mybir.AluOpType.subtract
    mult = mybir.AluOpType.mult
    add = mybir.AluOpType.add
    DR = mybir.MatmulPerfMode.DoubleRow

    nc = bacc.Bacc("TRN2", target_bir_lowering=False, debug=False, num_devices=NCORES)

    xt_d = nc.dram_tensor("xt", [B, D, T], f8, kind="ExternalInput")
    xres_d = nc.dram_tensor("xres", [4, P, D], f32, kind="ExternalInput")
    wq_d = nc.dram_tensor("wq", [D, CPC], f8, kind="ExternalInput")
    wk_d = nc.dram_tensor("wk", [B, D, CPC], f8, kind="ExternalInput")
    wv_d = nc.dram_tensor("wv", [D, CPC], f8, kind="ExternalInput")
    bq_d = nc.dram_tensor("bq", [B, CPC], f32, kind="ExternalInput")
    bk_d = nc.dram_tensor("bk", [B, CPC], f32, kind="ExternalInput")
    bv_d = nc.dram_tensor("bv", [B, CPC], f32, kind="ExternalInput")
    wo_d = nc.dram_tensor("wo", [D, D], bf, kind="ExternalInput")
    lng_d = nc.dram_tensor("lng", [D], f32, kind="ExternalInput")
    lnb_d = nc.dram_tensor("lnb", [D], f32, kind="ExternalInput")
    out_d = nc.dram_tensor("out", [4, P, D], f32, kind="ExternalOutput")
    a2ai_d = [nc.dram_tensor(f"a2ai{i}", [NCORES, CPC, B, 64], bf, kind="Internal")
              for i in range(4)]
    a2ao_d = [nc.dram_tensor(f"a2ao{i}", [NCORES, CPC, B, 64], bf, kind="Internal")
              for i in range(4)]
    rsd_d = [nc.dram_tensor(f"rsd{i}", [4, 512], f32, kind="Internal") for i in range(4)]

    def bcast_ap(src, nparts):
        """Prepend a stride-0 partition dim to a (DRAM) AP."""
        return bass.AP(
            tensor=src.tensor,
            offset=src.offset,
            ap=[[0, nparts]] + [list(d) for d in src.ap],
        )

    with tile.TileContext(nc) as tc:
        with tc.tile_pool(name="const", bufs=1) as const:
            qeng = [nc.sync, nc.scalar, nc.gpsimd]
            # fp8 x, DoubleRow pairing: element (p, kk) of chunk k2 is
            # input dim d = k2*256 + kk*128 + p
            x8_sb = [[const.tile([P, 2, T], f8, name=f"x8_{b}_{k2}")
                      for k2 in range(4)] for b in range(B)]
            wq8_sb = const.tile([P, 4, 2, CPC], f8)
            wk8_sb = const.tile([P, B, 4, 2, CPC], f8)
            wv8_sb = const.tile([P, 4, 2, CPC], f8)
            for k2 in range(4):
                qeng[k2 % 3].dma_start(
                    out=wq8_sb[:, k2, :, :],
                    in_=wq_d.ap()[k2 * 256:(k2 + 1) * 256, :]
                        .rearrange("(kk p) m -> p kk m", kk=2))
            for b in range(B):
                for k2 in range(4):
                    qeng[(b + k2) % 3].dma_start(
                        out=wk8_sb[:, b, k2, :, :],
                        in_=wk_d.ap()[b, k2 * 256:(k2 + 1) * 256, :]
                            .rearrange("(kk p) m -> p kk m", kk=2))
            for k2 in range(4):
                qeng[k2 % 3].dma_start(
                    out=wv8_sb[:, k2, :, :],
                    in_=wv_d.ap()[k2 * 256:(k2 + 1) * 256, :]
                        .rearrange("(kk p) m -> p kk m", kk=2))
            for b in range(B):
                for k2 in range(4):
                    qeng[(b * 4 + k2) % 3].dma_start(
                        out=x8_sb[b][k2][:],
                        in_=xt_d.ap()[b, k2 * 256:(k2 + 1) * 256, :]
                            .rearrange("(kk p) t -> p kk t", kk=2))

            wo_sb = const.tile([P, 8, D], bf)
            wo_ap = wo_d.ap()
            for k in range(8):
                qeng[k % 3].dma_start(out=wo_sb[:, k, :], in_=wo_ap[k * P:(k + 1) * P, :])

            bq_sb = const.tile([P, B], f32)
            bk_sb = const.tile([P, B], f32)
            nc.sync.dma_start(out=bq_sb[:], in_=bq_d.ap().rearrange("b p -> p b"))
            nc.sync.dma_start(out=bk_sb[:], in_=bk_d.ap().rearrange("b p -> p b"))
            bv_bc = const.tile([P, B, CPC], f32)
            for b in range(B):
                nc.sync.dma_start(out=bv_bc[:, b, :], in_=bcast_ap(bv_d.ap()[b, :], P))
            lng_b = const.tile([P, D], f32)
            lnb_b = const.tile([P, D], f32)
            nc.sync.dma_start(out=lng_b[:], in_=bcast_ap(lng_d.ap()[:], P))
            nc.sync.dma_start(out=lnb_b[:], in_=bcast_ap(lnb_d.ap()[:], P))
            xres_sb = const.tile([P, 4, D], f32)
            for qb in range(4):
                nc.scalar.dma_start(out=xres_sb[:, qb, :], in_=xres_d.ap()[qb])

            eps_sb = const.tile([P, 1], f32)
            nc.vector.memset(eps_sb[:], 1e-5)
            shift_sb = const.tile([P, 1], f32)
            nc.vector.memset(shift_sb[:], SHIFT)

            QT_sb = const.tile([P, B, T], bf)
            KT_sb = const.tile([P, B, T], bf)
            # V in fp8, DoubleRow layout: (scp key-pair, kk parity, hh head, 80)
            # col 64 is the ones column (AV row 64 = exp rowsum); cols 65-79
            # zero-pad for the DoubleRow 16B-stride rule.
            V8_sb = const.tile([P, B, 8, 2, 2, 80], f8)
            nc.vector.memset(V8_sb[:], 0.0)
            nc.vector.memset(V8_sb[:, :, :, :, :, 64:65], 1.0)
            attT_sb = [const.tile([64, B, 2, 512], bf, name=f"attT{i}")
                       for i in range(4)]

            # ---- phase 1: projections (all fp8 DoubleRow) ----
            with tc.tile_pool(name="qk_ps", bufs=4, space="PSUM") as qk_ps, \
                 tc.tile_pool(name="v_ps", bufs=4, space="PSUM") as v_ps:
                for wname, w8, b_sb, scale, dst in (
                        ("k", wk8_sb, bk_sb, 1.0 / WKV_SCALE, KT_sb),
                        ("q", wq8_sb, bq_sb, 1.0 / WQ_SCALE, QT_sb)):
                    for b in range(B):
                        for n in range(4):
                            ps = qk_ps.tile([P, 512], f32, tag="qkps")
                            for k2 in range(4):
                                lhsT = (w8[:, k2, :, :] if wname == "q"
                                        else w8[:, b, k2, :, :])
                                nc.tensor.matmul(
                                    ps[:], lhsT=lhsT,
                                    rhs=x8_sb[b][k2][:, :, n * 512:(n + 1) * 512],
                                    start=(k2 == 0), stop=(k2 == 3),
                                    perf_mode=DR,
                                )
                            nc.vector.tensor_scalar(
                                out=dst[:, b, n * 512:(n + 1) * 512],
                                in0=ps[:], scalar1=scale, scalar2=b_sb[:, b:b + 1],
                                op0=mult, op1=add,
                            )
                for b in range(B):
                    for mt in range(16):
                        ps = v_ps.tile([P, CPC], f32, tag="vps")
                        for k2 in range(4):
                            nc.tensor.matmul(
                                ps[:],
                                lhsT=x8_sb[b][k2][:, :, mt * P:(mt + 1) * P],
                                rhs=wv8_sb[:, k2, :, :],
                                start=(k2 == 0), stop=(k2 == 3),
                                perf_mode=DR,
                            )
                        nc.vector.scalar_tensor_tensor(
                            out=V8_sb[:, b, mt // 2, mt % 2, :, 0:64],
                            in0=ps[:].rearrange("p (h d) -> p h d", h=2),
                            scalar=1.0 / WKV_SCALE,
                            in1=bv_bc[:, b, :].rearrange("p (h d) -> p h d", h=2),
                            op0=mult, op1=add,
                        )

            # ---- phase 2: attention ----
            with tc.tile_pool(name="sp_ps", bufs=2, space="PSUM") as sp_ps, \
                 tc.tile_pool(name="av_ps", bufs=4, space="PSUM") as av_ps, \
                 tc.tile_pool(name="et_pool", bufs=3) as et_pool, \
                 tc.tile_pool(name="rc_pool", bufs=2) as rc_pool, \
                 tc.tile_pool(name="rb_pool", bufs=2) as rb_pool:
                for qb in range(4):
                    qsl = slice(qb * 512, (qb + 1) * 512)
                    avs = [av_ps.tile([80, 512], f32, tag="av", name=f"av{qb}_{u}")
                           for u in range(4)]
                    for b in range(B):
                        et = None
                        for sc in range(16):
                            sp = sp_ps.tile([P, 2, 512], f32, tag="sp")
                            nc.tensor.matmul(
                                sp[:, 0, :],
                                lhsT=KT_sb[0:64, b, sc * P:(sc + 1) * P],
                                rhs=QT_sb[0:64, b, qsl],
                                start=True, stop=True,
                            )
                            nc.tensor.matmul(
                                sp[:, 1, :],
                                lhsT=KT_sb[64:128, b, sc * P:(sc + 1) * P],
                                rhs=QT_sb[64:128, b, qsl],
                                start=True, stop=True,
                            )
                            if sc % 2 == 0:
                                et = et_pool.tile([P, 2, 2, 512], f8, tag="et")
                            nc.scalar.activation(
                                out=et[:, sc % 2, :, :], in_=sp[:],
                                func=Exp, bias=shift_sb[:],
                            )
                            if sc % 2 == 1:
                                for hh in range(2):
                                    nc.tensor.matmul(
                                        avs[2 * b + hh][:],
                                        lhsT=V8_sb[:, b, sc // 2, :, hh, :],
                                        rhs=et[:, :, hh, :],
                                        start=(sc == 1), stop=(sc == 15),
                                        perf_mode=DR,
                                    )
                        # per-(qb, b) epilogue half: the b0 half hides under
                        # b1's attention; only the (qb3, b1) half is exposed.
                        rs2 = rc_pool.tile([1, 2, 512], f32, tag="rs2",
                                           name=f"rs2_{qb}_{b}")
                        rc2 = rc_pool.tile([1, 2, 512], f32, tag="rc2",
                                           name=f"rc2_{qb}_{b}")
                        for hh in range(2):
                            nc.vector.tensor_copy(out=rs2[:, hh, :],
                                                  in_=avs[2 * b + hh][64:65, :])
                        # approx_fast needs SBUF input (PSUM reads give garbage)
                        nc.vector.reciprocal_approx_fast(out=rc2[:], in_=rs2[:])
                        nc.gpsimd.dma_start(
                            out=rsd_d[qb].ap()[2 * b:2 * b + 2, :], in_=rc2[:])
                        rcb = rb_pool.tile([64, 2, 512], f32, tag="rcb")
                        nc.sync.dma_start(
                            out=rcb[:],
                            in_=bcast_ap(rsd_d[qb].ap()[2 * b:2 * b + 2, :], 64))
                        for hh in range(2):
                            nc.vector.tensor_mul(
                                out=attT_sb[qb][:, b, hh, :],
                                in0=avs[2 * b + hh][0:64, :],
                                in1=rcb[:, hh, :],
                            )
                            nc.gpsimd.dma_start(
                                out=a2ai_d[qb].ap()[:, hh * 64:(hh + 1) * 64, b, :]
                                    .rearrange("j p i -> p j i"),
                                in_=attT_sb[qb][:, b, hh, :]
                                    .rearrange("p (j i) -> p j i", j=8),
                            )
                    nc.gpsimd.collective_compute(
                        "AllToAll",
                        mybir.AluOpType.bypass,
                        replica_groups=[list(range(NCORES))],
                        ins=[a2ai_d[qb].ap()],
                        outs=[a2ao_d[qb].ap()],
                    )

            # ---- phase 3: interleaved-row Wo + residual + LN per q-block ----
            with tc.tile_pool(name="wo_ps", bufs=2, space="PSUM") as wo_ps, \
                 tc.tile_pool(name="attf_pool", bufs=2) as attf_pool, \
                 tc.tile_pool(name="y_pool", bufs=2) as y_pool, \
                 tc.tile_pool(name="stat", bufs=4) as stat:
                for qb in range(4):
                    attf = attf_pool.tile([P, 8, B, 64], bf, tag="attf")
                    for k in range(8):
                        eng = nc.scalar if k % 2 == 0 else nc.sync
                        eng.dma_start(
                            out=attf[:, k, :, :],
                            in_=a2ao_d[qb].ap()[k][:, :, :],
                        )
                    y = y_pool.tile([P, D], f32, tag="y")
                    for n in range(2):
                        nsl = slice(n * 512, (n + 1) * 512)
                        ps = wo_ps.tile([P, 512], f32, tag="wops")
                        for k in range(8):
                            nc.tensor.matmul(
                                ps[:],
                                lhsT=attf[:, k, :, :],
                                rhs=wo_sb[:, k, nsl],
                                start=(k == 0), stop=(k == 7),
                            )
                        nc.vector.tensor_add(out=y[:, nsl], in0=ps[:],
                                             in1=xres_sb[:, qb, nsl])
                    st = stat.tile([P, 2, 6], f32, tag="st")
                    nc.vector.bn_stats(out=st[:, 0, :], in_=y[:, 0:512])
                    nc.vector.bn_stats(out=st[:, 1, :], in_=y[:, 512:1024])
                    mv = stat.tile([P, 2], f32, tag="mv")
                    nc.vector.bn_aggr(out=mv[:], in_=st[:])
                    std = stat.tile([P, 1], f32, tag="std")
                    nc.scalar.activation(out=std[:], in_=mv[:, 1:2], func=Sqrt, bias=eps_sb[:])
                    rstd = stat.tile([P, 1], f32, tag="rstd")
                    nc.vector.reciprocal(out=rstd[:], in_=std[:])
                    nc.vector.tensor_scalar(
                        out=y[:], in0=y[:], scalar1=mv[:, 0:1], scalar2=rstd[:],
                        op0=sub, op1=mult,
                    )
                    nc.gpsimd.tensor_mul(out=y[:], in0=y[:], in1=lng_b[:])
                    nc.gpsimd.tensor_add(out=y[:], in0=y[:], in1=lnb_b[:])
                    nc.sync.dma_start(out=out_d.ap()[qb], in_=y[:])

    nc.compile()
    return nc


def _prep_inputs(x, quantile, quantile_importance,
                 Wq, bq, Wk, bk, Wv, bv, Wo, bo,
                 qpq_w1, qpq_b1, qpq_w2, qpq_b2,
                 qpk_w1, qpk_b1, qpk_w2, qpk_b2,
                 qpv_w1, qpv_b1, qpv_w2, qpv_b2,
                 ln_g, ln_b):
    pe = _sinusoidal_pe(T, D)
    xp = x.astype(np.float32) + pe[None]

    q = quantile.astype(np.float32)

    def mlp(w1, b1, w2, b2):
        return np.maximum(q @ w1 + b1, 0.0) @ w2 + b2

    q_embed = mlp(qpq_w1, qpq_b1, qpq_w2, qpq_b2)
    k_embed = mlp(qpk_w1, qpk_b1, qpk_w2, qpk_b2)
    v_embed = mlp(qpv_w1, qpv_b1, qpv_w2, qpv_b2)

    buf = quantile_importance.astype(np.float32)
    idx = np.clip((q[:, 0] * 100).astype(np.int32), 0, 99)
    mx = buf.max()
    imp = buf[idx]
    imp = np.where(mx > 0, imp / mx, imp)
    scales = (1.0 + imp).astype(np.float32)          # [B], folded into Wk/bk
    rdk = np.float32(1.0 / np.sqrt(DK))              # folded into Wq/bq

    xt_all = np.ascontiguousarray(np.transpose(xp, (0, 2, 1))).astype(_F8)  # [B, D, T]
    xpb = xp + bo[None, None, :]
    in_maps = []
    for c in range(NCORES):
        cols = slice(c * CPC, (c + 1) * CPC)
        # block residual rows: xres[qb, b*64+i] = (x+pe+bo)[b, qb*512 + 64*c + i]
        xres = np.empty((4, P, D), np.float32)
        for qbi in range(4):
            for b in range(B):
                base = qbi * 512 + 64 * c
                xres[qbi, b * 64:(b + 1) * 64] = xpb[b, base:base + 64]
        in_maps.append({
            "xt": xt_all,
            "xres": xres,
            "wq": np.ascontiguousarray(Wq[:, cols] * (rdk * WQ_SCALE)).astype(_F8),
            "wk": np.ascontiguousarray(
                Wk[None, :, cols] * (scales[:, None, None] * WKV_SCALE)).astype(_F8),
            "wv": np.ascontiguousarray(Wv[:, cols] * WKV_SCALE).astype(_F8),
            "bq": np.ascontiguousarray((bq[None, cols] + q_embed[:, cols]) * rdk).astype(np.float32),
            "bk": np.ascontiguousarray((bk[None, cols] + k_embed[:, cols]) * scales[:, None]).astype(np.float32),
            "bv": np.ascontiguousarray(bv[None, cols] + v_embed[:, cols]).astype(np.float32),
            "wo": Wo.astype(_BF16),
            "lng": ln_g.astype(np.float32),
            "lnb": ln_b.astype(np.float32),
        })
    return in_maps


def kernel(**inputs):
    global LAST_RESULTS
    from concourse import bass_utils

    inputs = {k: np.asarray(v) for k, v in inputs.items()}
    if "nc" not in _CACHE:
        _CACHE["nc"] = _build()
    nc = _CACHE["nc"]

    in_maps = _prep_inputs(**inputs)
    res = bass_utils.run_bass_kernel_spmd(nc, in_maps, core_ids=list(range(NCORES)))
    LAST_RESULTS = res

    out = np.zeros((B, T, D), np.float32)
    for c in range(NCORES):
        o = res.results[c]["out"]  # [4, 128, D]; row = b*64+i
        for qbi in range(4):
            for b in range(B):
                base = qbi * 512 + 64 * c
                out[b, base:base + 64, :] = o[qbi, b * 64:(b + 1) * 64]
    return out


# revision 15
# speedup vs baseline: 1.5121x; 1.1998x over previous
"""Trainium2 Bass kernel for nn_AdaptiveAttention (8-core SPMD).

Sharding: each core owns 2 heads (one 128-dim block of the QKV/head space)
for BOTH batches; outputs are resharded in contiguous 64-row blocks: within
q-block qb, core c produces q-rows [qb*512 + 64*c, qb*512 + 64*(c+1)).

v3 highlights:
  - block (not interleaved) q-row ownership: normalize multiply and A2A
    pushes are fully contiguous, 4 big pushes per q-block.
  - x and the QKV weights travel in fp8e4 (weights host-prescaled by 64/16
    to dodge fp8 subnormals; the inverse scale rides the PSUM drain), and
    all three projections run DoubleRow (2 key-chunks per matmul pass).
    Halves both the x DMA lead-in (4MB) and the projection PE time.
  - exp() output is fp8e4 (scores shifted by -3.5; shift cancels in
    softmax); V fp8 padded to 80 dims (DoubleRow 16B-stride rule) with a
    ones column so AV row 64 is the exp row-sum; AV is DoubleRow too.
  - softmax reciprocal via reciprocal_approx_fast on an SBUF copy of the
    rowsum rows (PSUM input gives garbage; exact reciprocal on 1 partition
    costs 3.3us); partition-broadcast via one DRAM bounce per (qb, b).
  - epilogues are split per (qb, b) so the b0 half hides under b1's
    attention; one AllToAll per q-block (4 total) so cores re-align four
    times and only a 256KB exchange is exposed in the tail.
  - Scalar engine runs only the exps during attention (its 16.7M exps at
    128 lanes / 1.2GHz = ~118us are the floor); phase-3 LN tail ops are
    split between DVE and the Pool engine, attf loads use the then-idle
    scalar DMA queue.
"""

import numpy as np
import ml_dtypes

B, T, D = 2, 2048, 1024
H, DK = 16, 64
CPC = 128               # head-dim columns per core (2 heads)
P = 128
NCORES = 8
SHIFT = -3.5            # exp shift: scores bounded ~[-8.3, 7.9]
WQ_SCALE = 64.0         # host premultiplier on wq (fp8 subnormal dodge)
WKV_SCALE = 16.0        # host premultiplier on wk/wv

_BF16 = ml_dtypes.bfloat16
_F8 = ml_dtypes.float8_e4m3

_CACHE = {}
LAST_RESULTS = None


def _sinusoidal_pe(max_len, d_model):
    pos = np.arange(max_len)[:, None].astype(np.float32)
    div = np.exp(np.arange(0, d_model, 2).astype(np.float32) * (-np.log(10000.0) / d_model))
    pe = np.zeros((max_len, d_model), dtype=np.float32)
    pe[:, 0::2] = np.sin(pos * div)
    pe[:, 1::2] = np.cos(pos * div)
    return pe


def _build():
    """Build + compile the SPMD Bass graph (one NEFF, runs on all 8 cores)."""
    import concourse.bass as bass
    import concourse.mybir as mybir
    import concourse.tile as tile
    from concourse import bacc

    f32 = mybir.dt.float32
    bf = mybir.dt.bfloat16
    f8 = mybir.dt.float8e4
    Exp = mybir.ActivationFunctionType.Exp
    Sqrt = mybir.ActivationFunctionType.Sqrt
    sub = mybir.AluOpType.subtract
    mult = mybir.AluOpType.mult
    add = mybir.AluOpType.add
    DR = mybir.MatmulPerfMode.DoubleRow

    nc = bacc.Bacc("TRN2", target_bir_lowering=False, debug=False, num_devices=NCORES)

    xt_d = nc.dram_tensor("xt", [B, D, T], f8, kind="ExternalInput")
    xres_d = nc.dram_tensor("xres", [4, P, D], f32, kind="ExternalInput")
    wq_d = nc.dram_tensor("wq", [D, CPC], f8, kind="ExternalInput")
    wk_d = nc.dram_tensor("wk", [B, D, CPC], f8, kind="ExternalInput")
    wv_d = nc.dram_tensor("wv", [D, CPC], f8, kind="ExternalInput")
    bq_d = nc.dram_tensor("bq", [B, CPC], f32, kind="ExternalInput")
    bk_d = nc.dram_tensor("bk", [B, CPC], f32, kind="ExternalInput")
    bv_d = nc.dram_tensor("bv", [B, CPC], f32, kind="ExternalInput")
    wo_d = nc.dram_tensor("wo", [D, D], bf, kind="ExternalInput")
    lng_d = nc.dram_tensor("lng", [D], f32, kind="ExternalInput")
    lnb_d = nc.dram_tensor("lnb", [D], f32, kind="ExternalInput")
    out_d = nc.dram_tensor("out", [4, P, D], f32, kind="ExternalOutput")
    a2ai_d = [nc.dram_tensor(f"a2ai{i}", [NCORES, CPC, B, 64], bf, kind="Internal")
              for i in range(4)]
    a2ao_d = [nc.dram_tensor(f"a2ao{i}", [NCORES, CPC, B, 64], bf, kind="Internal")
              for i in range(4)]
    rsd_d = [nc.dram_tensor(f"rsd{i}", [4, 512], f32, kind="Internal") for i in range(4)]

    def bcast_ap(src, nparts):
        """Prepend a stride-0 partition dim to a (DRAM) AP."""
        return bass.AP(
            tensor=src.tensor,
            offset=src.offset,
            ap=[[0, nparts]] + [list(d) for d in src.ap],
        )

    with tile.TileContext(nc) as tc:
        with tc.tile_pool(name="const", bufs=1) as const:
            qeng = [nc.sync, nc.scalar, nc.gpsimd]
            # fp8 x, DoubleRow pairing: element (p, kk) of chunk k2 is
            # input dim d = k2*256 + kk*128 + p
            x8_sb = [[const.tile([P, 2, T], f8, name=f"x8_{b}_{k2}")
                      for k2 in range(4)] for b in range(B)]
            wq8_sb = const.tile([P, 4, 2, CPC], f8)
            wk8_sb = const.tile([P, B, 4, 2, CPC], f8)
            wv8_sb = const.tile([P, 4, 2, CPC], f8)

            def load_x8(b, k2, eng):
                eng.dma_start(
                    out=x8_sb[b][k2][:],
                    in_=xt_d.ap()[b, k2 * 256:(k2 + 1) * 256, :]
                        .rearrange("(kk p) t -> p kk t", kk=2))

            def load_w8(dst, src, k2, eng):
                eng.dma_start(
                    out=dst, in_=src[k2 * 256:(k2 + 1) * 256, :]
                        .rearrange("(kk p) m -> p kk m", kk=2))

            # attention-critical loads first: x8(b0), wk(b0), wq --> K/Q(b0)
            # can start ~4us in; everything phase-3-only (wo, xres, ln) last.
            for k2 in range(4):
                load_x8(0, k2, qeng[k2 % 3])
            for k2 in range(4):
                load_w8(wk8_sb[:, 0, k2, :, :], wk_d.ap()[0], k2, qeng[k2 % 3])
            for k2 in range(4):
                load_w8(wq8_sb[:, k2, :, :], wq_d.ap(), k2, qeng[(k2 + 1) % 3])
            bq_sb = const.tile([P, B], f32)
            bk_sb = const.tile([P, B], f32)
            nc.sync.dma_start(out=bq_sb[:], in_=bq_d.ap().rearrange("b p -> p b"))
            nc.sync.dma_start(out=bk_sb[:], in_=bk_d.ap().rearrange("b p -> p b"))
            for k2 in range(4):
                load_x8(1, k2, qeng[k2 % 3])
            for k2 in range(4):
                load_w8(wk8_sb[:, 1, k2, :, :], wk_d.ap()[1], k2, qeng[(k2 + 1) % 3])
            for k2 in range(4):
                load_w8(wv8_sb[:, k2, :, :], wv_d.ap(), k2, qeng[(k2 + 2) % 3])
            bv_bc = const.tile([P, B, CPC], f32)
            for b in range(B):
                nc.sync.dma_start(out=bv_bc[:, b, :], in_=bcast_ap(bv_d.ap()[b, :], P))

            wo_sb = const.tile([P, 8, D], bf)
            wo_ap = wo_d.ap()
            for k in range(8):
                qeng[k % 3].dma_start(out=wo_sb[:, k, :], in_=wo_ap[k * P:(k + 1) * P, :])
            lng_b = const.tile([P, D], f32)
            lnb_b = const.tile([P, D], f32)
            nc.sync.dma_start(out=lng_b[:], in_=bcast_ap(lng_d.ap()[:], P))
            nc.sync.dma_start(out=lnb_b[:], in_=bcast_ap(lnb_d.ap()[:], P))
            xres_sb = const.tile([P, 4, D], f32)
            for qb in range(4):
                nc.scalar.dma_start(out=xres_sb[:, qb, :], in_=xres_d.ap()[qb])

            eps_sb = const.tile([P, 1], f32)
            nc.vector.memset(eps_sb[:], 1e-5)
            shift_sb = const.tile([P, 1], f32)
            nc.vector.memset(shift_sb[:], SHIFT)

            QT_sb = const.tile([P, B, T], bf)
            KT_sb = const.tile([P, B, T], bf)
            # V in fp8, DoubleRow layout: (scp key-pair, kk parity, hh head, 80)
            # col 64 is the ones column (AV row 64 = exp rowsum); cols 65-79
            # zero-pad for the DoubleRow 16B-stride rule.
            V8_sb = const.tile([P, B, 8, 2, 2, 80], f8)
            nc.vector.memset(V8_sb[:], 0.0)
            nc.vector.memset(V8_sb[:, :, :, :, :, 64:65], 1.0)
            attT_sb = [const.tile([64, B, 2, 512], bf, name=f"attT{i}")
                       for i in range(4)]

            # ---- phase 1: projections (all fp8 DoubleRow) ----
            with tc.tile_pool(name="qk_ps", bufs=4, space="PSUM") as qk_ps, \
                 tc.tile_pool(name="v_ps", bufs=4, space="PSUM") as v_ps:
                for wname, w8, b_sb, scale, dst in (
                        ("k", wk8_sb, bk_sb, 1.0 / WKV_SCALE, KT_sb),
                        ("q", wq8_sb, bq_sb, 1.0 / WQ_SCALE, QT_sb)):
                    for b in range(B):
                        for n in range(4):
                            ps = qk_ps.tile([P, 512], f32, tag="qkps")
                            for k2 in range(4):
                                lhsT = (w8[:, k2, :, :] if wname == "q"
                                        else w8[:, b, k2, :, :])
                                nc.tensor.matmul(
                                    ps[:], lhsT=lhsT,
                                    rhs=x8_sb[b][k2][:, :, n * 512:(n + 1) * 512],
                                    start=(k2 == 0), stop=(k2 == 3),
                                    perf_mode=DR,
                                )
                            nc.vector.tensor_scalar(
                                out=dst[:, b, n * 512:(n + 1) * 512],
                                in0=ps[:], scalar1=scale, scalar2=b_sb[:, b:b + 1],
                                op0=mult, op1=add,
                            )
                for b in range(B):
                    for mt in range(16):
                        ps = v_ps.tile([P, CPC], f32, tag="vps")
                        for k2 in range(4):
                            nc.tensor.matmul(
                                ps[:],
                                lhsT=x8_sb[b][k2][:, :, mt * P:(mt + 1) * P],
                                rhs=wv8_sb[:, k2, :, :],
                                start=(k2 == 0), stop=(k2 == 3),
                                perf_mode=DR,
                            )
                        nc.vector.scalar_tensor_tensor(
                            out=V8_sb[:, b, mt // 2, mt % 2, :, 0:64],
                            in0=ps[:].rearrange("p (h d) -> p h d", h=2),
                            scalar=1.0 / WKV_SCALE,
                            in1=bv_bc[:, b, :].rearrange("p (h d) -> p h d", h=2),
                            op0=mult, op1=add,
                        )

            # ---- phase 2: attention ----
            with tc.tile_pool(name="sp_ps", bufs=2, space="PSUM") as sp_ps, \
                 tc.tile_pool(name="av_ps", bufs=4, space="PSUM") as av_ps, \
                 tc.tile_pool(name="et_pool", bufs=3) as et_pool, \
                 tc.tile_pool(name="rc_pool", bufs=2) as rc_pool, \
                 tc.tile_pool(name="rb_pool", bufs=2) as rb_pool:
                for qb in range(4):
                    qsl = slice(qb * 512, (qb + 1) * 512)
                    avs = [av_ps.tile([80, 512], f32, tag="av", name=f"av{qb}_{u}")
                           for u in range(4)]
                    for b in range(B):
                        # software-pipelined: AV of pair i is emitted AFTER the
                        # scores of pair i+1, so the PE never sits behind an
                        # exp-dependent AV while scores could run.
                        ets = {}

                        def emit_av(i):
                            for hh in range(2):
                                nc.tensor.matmul(
                                    avs[2 * b + hh][:],
                                    lhsT=V8_sb[:, b, i, :, hh, :],
                                    rhs=ets[i][:, :, hh, :],
                                    start=(i == 0), stop=(i == 7),
                                    perf_mode=DR,
                                )
                            del ets[i]

                        for scp in range(8):
                            for par in range(2):
                                sc = 2 * scp + par
                                sp = sp_ps.tile([P, 2, 512], f32, tag="sp")
                                nc.tensor.matmul(
                                    sp[:, 0, :],
                                    lhsT=KT_sb[0:64, b, sc * P:(sc + 1) * P],
                                    rhs=QT_sb[0:64, b, qsl],
                                    start=True, stop=True,
                                )
                                nc.tensor.matmul(
                                    sp[:, 1, :],
                                    lhsT=KT_sb[64:128, b, sc * P:(sc + 1) * P],
                                    rhs=QT_sb[64:128, b, qsl],
                                    start=True, stop=True,
                                )
                                if par == 0:
                                    ets[scp] = et_pool.tile(
                                        [P, 2, 2, 512], f8, tag="et",
                                        name=f"et{qb}_{b}_{scp}")
                                nc.scalar.activation(
                                    out=ets[scp][:, par, :, :], in_=sp[:],
                                    func=Exp, bias=shift_sb[:],
                                )
                            if scp >= 1:
                                emit_av(scp - 1)
                        emit_av(7)
                        # per-(qb, b) epilogue half: the b0 half hides under
                        # b1's attention; only the (qb3, b1) half is exposed.
                        rs2 = rc_pool.tile([1, 2, 512], f32, tag="rs2",
                                           name=f"rs2_{qb}_{b}")
                        rc2 = rc_pool.tile([1, 2, 512], f32, tag="rc2",
                                           name=f"rc2_{qb}_{b}")
                        for hh in range(2):
                            nc.vector.tensor_copy(out=rs2[:, hh, :],
                                                  in_=avs[2 * b + hh][64:65, :])
                        # approx_fast needs SBUF input (PSUM reads give garbage)
                        nc.vector.reciprocal_approx_fast(out=rc2[:], in_=rs2[:])
                        nc.gpsimd.dma_start(
                            out=rsd_d[qb].ap()[2 * b:2 * b + 2, :], in_=rc2[:])
                        rcb = rb_pool.tile([64, 2, 512], f32, tag="rcb")
                        nc.sync.dma_start(
                            out=rcb[:],
                            in_=bcast_ap(rsd_d[qb].ap()[2 * b:2 * b + 2, :], 64))
                        for hh in range(2):
                            nc.vector.tensor_mul(
                                out=attT_sb[qb][:, b, hh, :],
                                in0=avs[2 * b + hh][0:64, :],
                                in1=rcb[:, hh, :],
                            )
                            nc.gpsimd.dma_start(
                                out=a2ai_d[qb].ap()[:, hh * 64:(hh + 1) * 64, b, :]
                                    .rearrange("j p i -> p j i"),
                                in_=attT_sb[qb][:, b, hh, :]
                                    .rearrange("p (j i) -> p j i", j=8),
                            )
                    nc.gpsimd.collective_compute(
                        "AllToAll",
                        mybir.AluOpType.bypass,
                        replica_groups=[list(range(NCORES))],
                        ins=[a2ai_d[qb].ap()],
                        outs=[a2ao_d[qb].ap()],
                    )

            # ---- phase 3: interleaved-row Wo + residual + LN per q-block ----
            with tc.tile_pool(name="wo_ps", bufs=2, space="PSUM") as wo_ps, \
                 tc.tile_pool(name="attf_pool", bufs=2) as attf_pool, \
                 tc.tile_pool(name="y_pool", bufs=2) as y_pool, \
                 tc.tile_pool(name="stat", bufs=4) as stat:
                for qb in range(4):
                    attf = attf_pool.tile([P, 8, B, 64], bf, tag="attf")
                    for k in range(8):
                        eng = nc.scalar if k % 2 == 0 else nc.sync
                        eng.dma_start(
                            out=attf[:, k, :, :],
                            in_=a2ao_d[qb].ap()[k][:, :, :],
                        )
                    y = y_pool.tile([P, D], f32, tag="y")
                    for n in range(2):
                        nsl = slice(n * 512, (n + 1) * 512)
                        ps = wo_ps.tile([P, 512], f32, tag="wops")
                        for k in range(8):
                            nc.tensor.matmul(
                                ps[:],
                                lhsT=attf[:, k, :, :],
                                rhs=wo_sb[:, k, nsl],
                                start=(k == 0), stop=(k == 7),
                            )
                        nc.vector.tensor_add(out=y[:, nsl], in0=ps[:],
                                             in1=xres_sb[:, qb, nsl])
                    st = stat.tile([P, 2, 6], f32, tag="st")
                    nc.vector.bn_stats(out=st[:, 0, :], in_=y[:, 0:512])
                    nc.vector.bn_stats(out=st[:, 1, :], in_=y[:, 512:1024])
                    mv = stat.tile([P, 2], f32, tag="mv")
                    nc.vector.bn_aggr(out=mv[:], in_=st[:])
                    std = stat.tile([P, 1], f32, tag="std")
                    nc.scalar.activation(out=std[:], in_=mv[:, 1:2], func=Sqrt, bias=eps_sb[:])
                    rstd = stat.tile([P, 1], f32, tag="rstd")
                    nc.vector.reciprocal(out=rstd[:], in_=std[:])
                    nc.vector.tensor_scalar(
                        out=y[:], in0=y[:], scalar1=mv[:, 0:1], scalar2=rstd[:],
                        op0=sub, op1=mult,
                    )
                    # qb0-2 tails go to the Pool engine (DVE is the phase-3
                    # bottleneck); qb3 is the exposed critical path, where
                    # DVE's higher throughput wins.
                    eng = nc.vector if qb == 3 else nc.gpsimd
                    eng.tensor_mul(out=y[:], in0=y[:], in1=lng_b[:])
                    eng.tensor_add(out=y[:], in0=y[:], in1=lnb_b[:])
                    nc.sync.dma_start(out=out_d.ap()[qb], in_=y[:])

    nc.compile()
    return nc


def _prep_inputs(x, quantile, quantile_importance,
                 Wq, bq, Wk, bk, Wv, bv, Wo, bo,
                 qpq_w1, qpq_b1, qpq_w2, qpq_b2,
                 qpk_w1, qpk_b1, qpk_w2, qpk_b2,
                 qpv_w1, qpv_b1, qpv_w2, qpv_b2,
                 ln_g, ln_b):
    pe = _sinusoidal_pe(T, D)
    xp = x.astype(np.float32) + pe[None]

    q = quantile.astype(np.float32)

    def mlp(w1, b1, w2, b2):
        return np.maximum(q @ w1 + b1, 0.0) @ w2 + b2

    q_embed = mlp(qpq_w1, qpq_b1, qpq_w2, qpq_b2)
    k_embed = mlp(qpk_w1, qpk_b1, qpk_w2, qpk_b2)
    v_embed = mlp(qpv_w1, qpv_b1, qpv_w2, qpv_b2)

    buf = quantile_importance.astype(np.float32)
    idx = np.clip((q[:, 0] * 100).astype(np.int32), 0, 99)
    mx = buf.max()
    imp = buf[idx]
    imp = np.where(mx > 0, imp / mx, imp)
    scales = (1.0 + imp).astype(np.float32)          # [B], folded into Wk/bk
    rdk = np.float32(1.0 / np.sqrt(DK))              # folded into Wq/bq

    xt_all = np.ascontiguousarray(np.transpose(xp, (0, 2, 1))).astype(_F8)  # [B, D, T]
    xpb = xp + bo[None, None, :]
    in_maps = []
    for c in range(NCORES):
        cols = slice(c * CPC, (c + 1) * CPC)
        # block residual rows: xres[qb, b*64+i] = (x+pe+bo)[b, qb*512 + 64*c + i]
        xres = np.empty((4, P, D), np.float32)
        for qbi in range(4):
            for b in range(B):
                base = qbi * 512 + 64 * c
                xres[qbi, b * 64:(b + 1) * 64] = xpb[b, base:base + 64]
        in_maps.append({
            "xt": xt_all,
            "xres": xres,
            "wq": np.ascontiguousarray(Wq[:, cols] * (rdk * WQ_SCALE)).astype(_F8),
            "wk": np.ascontiguousarray(
                Wk[None, :, cols] * (scales[:, None, None] * WKV_SCALE)).astype(_F8),
            "wv": np.ascontiguousarray(Wv[:, cols] * WKV_SCALE).astype(_F8),
            "bq": np.ascontiguousarray((bq[None, cols] + q_embed[:, cols]) * rdk).astype(np.float32),
            "bk": np.ascontiguousarray((bk[None, cols] + k_embed[:, cols]) * scales[:, None]).astype(np.float32),
            "bv": np.ascontiguousarray(bv[None, cols] + v_embed[:, cols]).astype(np.float32),
            "wo": Wo.astype(_BF16),
            "lng": ln_g.astype(np.float32),
            "lnb": ln_b.astype(np.float32),
        })
    return in_maps


def kernel(**inputs):
    global LAST_RESULTS
    from concourse import bass_utils

    inputs = {k: np.asarray(v) for k, v in inputs.items()}
    if "nc" not in _CACHE:
        _CACHE["nc"] = _build()
    nc = _CACHE["nc"]

    in_maps = _prep_inputs(**inputs)
    res = bass_utils.run_bass_kernel_spmd(nc, in_maps, core_ids=list(range(NCORES)))
    LAST_RESULTS = res

    out = np.zeros((B, T, D), np.float32)
    for c in range(NCORES):
        o = res.results[c]["out"]  # [4, 128, D]; row = b*64+i
        for qbi in range(4):
            for b in range(B):
                base = qbi * 512 + 64 * c
                out[b, base:base + 64, :] = o[qbi, b * 64:(b + 1) * 64]
    return out
